# revision 1
# baseline (speedup 1.0000x reference)
"""MoE (dense all-expert FFN with double-softmax routing) on 8 trn2 NeuronCores.

Expert-parallel: core c holds expert c's W1/W2/b1/b2 resident in SBUF (bf16)
and computes its expert's routing-weighted contribution
    contrib_c = weight_c * mask_c * (swish(x @ W1[c] + b1[c]) @ W2[c] + b2[c])
for all 4096 tokens, written transposed as [1024, 4096].  The host gathers the
8 partial outputs and forms  sum_c(contrib_c)^T + x  (a pure 8-way reduction +
residual + layout transform; all matmuls / softmaxes / activations / masking
run on device).

All device tensors live transposed ([feature, token]) so the contraction dim
is on SBUF partitions for every matmul.  Host prep is layout/dtype only
(transpose + bf16 cast + per-expert slicing).
"""

import os
import numpy as np
import ml_dtypes

B, D, E, U = 4096, 1024, 8, 4096
BT = 512              # token tile (matmul free dim)
NB = B // BT          # 8 token tiles
DC = D // 128         # 8 chunks of the model dim
UC = U // 128         # 32 chunks of the hidden dim
N_CORES = 8
P = 128

_BF16 = ml_dtypes.bfloat16

_NC_CACHE = {}
LAST_RESULTS = None


def _build_nc(bench_loop=0):
    import concourse.mybir as mybir
    import concourse.tile as tile
    from concourse import bacc

    f32 = mybir.dt.float32
    bf16 = mybir.dt.bfloat16
    AF = mybir.ActivationFunctionType
    ALU = mybir.AluOpType

    nc = bacc.Bacc("TRN2", target_bir_lowering=False, debug=False,
                   num_devices=N_CORES)

    xt = nc.dram_tensor("xt", [D, B], bf16, kind="ExternalInput").ap()
    w1 = nc.dram_tensor("w1", [D, U], bf16, kind="ExternalInput").ap()
    w2 = nc.dram_tensor("w2", [U, D], bf16, kind="ExternalInput").ap()
    wr = nc.dram_tensor("wr", [D, E], bf16, kind="ExternalInput").ap()
    br = nc.dram_tensor("br", [E, 1], f32, kind="ExternalInput").ap()
    b1 = nc.dram_tensor("b1", [P, UC], f32, kind="ExternalInput").ap()
    b2 = nc.dram_tensor("b2", [P, DC], f32, kind="ExternalInput").ap()
    selb = nc.dram_tensor("selb", [E, P], f32, kind="ExternalInput").ap()
    ones8 = nc.dram_tensor("ones8", [E, 1], f32, kind="ExternalInput").ap()
    ones18 = nc.dram_tensor("ones18", [1, E], f32, kind="ExternalInput").ap()
    o = nc.dram_tensor("o", [D, B], f32, kind="ExternalOutput").ap()

    with tile.TileContext(nc) as tc:
        with (
            tc.tile_pool(name="wp", bufs=1) as wp,
            tc.tile_pool(name="xbp", bufs=17) as xbp,
            tc.tile_pool(name="hbp", bufs=34) as hbp,
            tc.tile_pool(name="r8p", bufs=4) as r8p,
            tc.tile_pool(name="r1p", bufs=2) as r1p,
            tc.tile_pool(name="scp", bufs=2) as scp,
            tc.tile_pool(name="ctp", bufs=4) as ctp,
            tc.tile_pool(name="ps1p", bufs=2, space="PSUM") as ps1p,
            tc.tile_pool(name="ps2p", bufs=2, space="PSUM") as ps2p,
            tc.tile_pool(name="psrp", bufs=2, space="PSUM") as psrp,
            tc.tile_pool(name="pssp", bufs=1, space="PSUM") as pssp,
        ):
            w1t = wp.tile([P, DC * U], bf16)
            w2t = wp.tile([P, UC * D], bf16)
            wrt = wp.tile([P, DC * E], bf16)
            b1t = wp.tile([P, UC], f32)
            b2t = wp.tile([P, DC], f32)
            brt = wp.tile([E, 1], f32)
            selbt = wp.tile([E, P], f32)
            o8t = wp.tile([E, 1], f32)
            o18t = wp.tile([1, E], f32)

            def emit_xb(bt):
                b0 = bt * BT
                xbs = []
                for dc in range(DC):
                    xb = xbp.tile([P, BT], bf16, tag="xb")
                    nc.sync.dma_start(
                        out=xb[:], in_=xt[dc * P : (dc + 1) * P, b0 : b0 + BT]
                    )
                    xbs.append(xb)
                return xbs

            def emit_router(xbs):
                # weights = softmax(softmax(x@Wr + br)), gate >0.1, row e
                # broadcast to 128 partitions
                lg = psrp.tile([E, BT], f32, tag="rps")
                for dc in range(DC):
                    nc.tensor.matmul(
                        lg[:], wrt[:, dc * E : (dc + 1) * E], xbs[dc][:],
                        start=(dc == 0), stop=(dc == DC - 1),
                    )
                t1 = r8p.tile([E, BT], f32, tag="r8")
                nc.scalar.activation(t1[:], lg[:], AF.Exp, bias=brt[:, 0:1])
                s1 = psrp.tile([1, BT], f32, tag="rps")
                nc.tensor.matmul(s1[:], o8t[:], t1[:], start=True, stop=True)
                r1 = r1p.tile([1, BT], f32, tag="r1")
                nc.vector.reciprocal(r1[:], s1[:])
                rb1 = psrp.tile([E, BT], f32, tag="rps")
                nc.tensor.matmul(rb1[:], o18t[:], r1[:], start=True, stop=True)
                pp = r8p.tile([E, BT], f32, tag="r8")
                nc.vector.tensor_tensor(pp[:], t1[:], rb1[:], ALU.mult)
                t2 = r8p.tile([E, BT], f32, tag="r8")
                nc.scalar.activation(t2[:], pp[:], AF.Exp)
                s2 = psrp.tile([1, BT], f32, tag="rps")
                nc.tensor.matmul(s2[:], o8t[:], t2[:], start=True, stop=True)
                r2 = r1p.tile([1, BT], f32, tag="r1")
                nc.vector.reciprocal(r2[:], s2[:])
                rb2 = psrp.tile([E, BT], f32, tag="rps")
                nc.tensor.matmul(rb2[:], o18t[:], r2[:], start=True, stop=True)
                wg = r8p.tile([E, BT], f32, tag="r8")
                nc.vector.tensor_tensor(wg[:], t2[:], rb2[:], ALU.mult)
                sc = r8p.tile([E, BT], f32, tag="r8")
                nc.vector.scalar_tensor_tensor(
                    sc[:], wg[:], 0.1, wg[:], ALU.is_gt, ALU.mult
                )
                s128ps = pssp.tile([P, BT], f32, tag="pss")
                nc.tensor.matmul(s128ps[:], selbt[:], sc[:], start=True, stop=True)
                s128 = scp.tile([P, BT], f32, tag="s128")
                nc.vector.tensor_copy(s128[:], s128ps[:])
                return s128

            # small/const DMAs first so the bt=0 router isn't queued behind
            # the 16.8MB of weights
            nc.sync.dma_start(out=b1t[:], in_=b1[:])
            nc.sync.dma_start(out=b2t[:], in_=b2[:])
            nc.sync.dma_start(out=brt[:], in_=br[:])
            nc.sync.dma_start(out=selbt[:], in_=selb[:])
            nc.sync.dma_start(out=o8t[:], in_=ones8[:])
            nc.sync.dma_start(out=o18t[:], in_=ones18[:])
            for dc in range(DC):
                nc.sync.dma_start(
                    out=wrt[:, dc * E : (dc + 1) * E],
                    in_=wr[dc * P : (dc + 1) * P, :],
                )
            def emit_main(prologue=None):
                if prologue is None:
                    xbs_cur = emit_xb(0)
                    s128_cur = emit_router(xbs_cur)
                else:
                    xbs_cur, s128_cur = prologue
                for bt in range(NB):
                    b0 = bt * BT
                    xbs = xbs_cur
                    s128 = s128_cur

                    # ---- h^T = swish(W1^T x^T + b1), bf16 chunks on U
                    hbs = []
                    for uc in range(UC):
                        ps1 = ps1p.tile([P, BT], f32, tag="ps1")
                        for dc in range(DC):
                            nc.tensor.matmul(
                                ps1[:],
                                w1t[:, dc * U + uc * P : dc * U + (uc + 1) * P],
                                xbs[dc][:],
                                start=(dc == 0), stop=(dc == DC - 1),
                            )
                        hb = hbp.tile([P, BT], bf16, tag="hb")
                        nc.scalar.activation(hb[:], ps1[:], AF.Silu,
                                             bias=b1t[:, uc : uc + 1])
                        hbs.append(hb)

                    # prefetch + route the next token tile while this computes
                    if bt + 1 < NB:
                        xbs_cur = emit_xb(bt + 1)
                        s128_cur = emit_router(xbs_cur)

                    # ---- contrib^T = (W2^T h^T + b2) * scale  -> DRAM
                    for dc in range(DC):
                        ps2 = ps2p.tile([P, BT], f32, tag="ps2")
                        for uc in range(UC):
                            nc.tensor.matmul(
                                ps2[:],
                                w2t[:, uc * D + dc * P : uc * D + (dc + 1) * P],
                                hbs[uc][:],
                                start=(uc == 0), stop=(uc == UC - 1),
                            )
                        ct = ctp.tile([P, BT], f32, tag="ct")
                        nc.vector.scalar_tensor_tensor(
                            ct[:], ps2[:], b2t[:, dc : dc + 1], s128[:],
                            ALU.add, ALU.mult,
                        )
                        nc.sync.dma_start(
                            out=o[dc * P : (dc + 1) * P, b0 : b0 + BT], in_=ct[:]
                        )

            if not bench_loop:
                xbs0 = emit_xb(0)
                s1280 = emit_router(xbs0)

            # W1 in uc-consumption order (g-major) so mm1 can start early;
            # W2 after (first needed ~100us in)
            for g in range(4):
                for dc in range(DC):
                    nc.sync.dma_start(
                        out=w1t[:, dc * U + g * 1024 : dc * U + (g + 1) * 1024],
                        in_=w1[dc * P : (dc + 1) * P, g * 1024 : (g + 1) * 1024],
                    )
            for uc in range(UC):
                nc.sync.dma_start(
                    out=w2t[:, uc * D : (uc + 1) * D],
                    in_=w2[uc * P : (uc + 1) * P, :],
                )

            if bench_loop:
                with tc.For_i(0, bench_loop, 1):
                    emit_main()
            else:
                emit_main(prologue=(xbs0, s1280))

    nc.compile()
    return nc


def _get_nc():
    if "nc" not in _NC_CACHE:
        _NC_CACHE["nc"] = _build_nc()
    return _NC_CACHE["nc"]


def _prep_in_maps(inputs):
    x = np.asarray(inputs["x"], np.float32)
    Wr = np.asarray(inputs["Wr"], np.float32)
    br = np.asarray(inputs["br"], np.float32)
    W1 = np.asarray(inputs["W1"], np.float32)
    b1 = np.asarray(inputs["b1"], np.float32)
    W2 = np.asarray(inputs["W2"], np.float32)
    b2 = np.asarray(inputs["b2"], np.float32)

    xT = np.ascontiguousarray(x.T)            # [D, B] f32
    xt_bf = xT.astype(_BF16)
    wr_bf = np.ascontiguousarray(Wr).astype(_BF16)
    br_c = np.ascontiguousarray(br.reshape(E, 1))
    ones8_c = np.ones((E, 1), np.float32)
    ones18_c = np.ones((1, E), np.float32)

    in_maps = []
    for c in range(N_CORES):
        sel = np.zeros((E, P), np.float32)
        sel[c, :] = 1.0
        in_maps.append({
            "xt": xt_bf,
            "w1": np.ascontiguousarray(W1[c]).astype(_BF16),
            "w2": np.ascontiguousarray(W2[c]).astype(_BF16),
            "wr": wr_bf,
            "br": br_c,
            "b1": np.ascontiguousarray(b1[c].reshape(UC, P).T),
            "b2": np.ascontiguousarray(b2[c].reshape(DC, P).T),
            "selb": sel,
            "ones8": ones8_c,
            "ones18": ones18_c,
        })
    return in_maps


def kernel(**inputs):
    from concourse.bass_utils import run_bass_kernel_spmd

    global LAST_RESULTS

    in_maps = _prep_in_maps(inputs)
    nc = _get_nc()
    want_trace = bool(int(os.environ.get("KERNEL_TRACE", "0")))
    if not want_trace:
        # the NTFF-trace path needs antenv.axon_hooks, which this container
        # lacks; make sure a stray BASS_TRACE env can't route us into it
        os.environ["BASS_NEVER_TRACE"] = "1"
    res = run_bass_kernel_spmd(
        nc, in_maps, core_ids=list(range(N_CORES)), trace=want_trace,
    )
    LAST_RESULTS = res

    # host: 8-way partial-sum reduction + residual + transpose back
    acc = res.results[0]["o"].astype(np.float32, copy=True)
    for c in range(1, N_CORES):
        acc += res.results[c]["o"]
    out = acc.T + np.asarray(inputs["x"], np.float32)
    return np.ascontiguousarray(out)



# revision 2
# speedup vs baseline: 1.7893x; 1.7893x over previous
"""MoE (dense all-expert FFN with double-softmax routing) on 8 trn2 NeuronCores.

Expert-parallel: core c holds expert c's W1/W2/b1/b2 resident in SBUF (fp8e4,
pre-scaled x64 on host for precision) and computes its expert's routing-
weighted contribution
    contrib_c = weight_c * mask_c * (swish(x @ W1[c] + b1[c]) @ W2[c] + b2[c])
for all 4096 tokens, written transposed as [1024, 4096].  The host gathers the
8 partial outputs and forms  sum_c(contrib_c)^T + x  (a pure 8-way reduction +
residual + layout transform; all matmuls / softmaxes / activations / masking
run on device).

Both big matmuls run in fp8 DoubleRow mode (2 fp8 weights per PE cell,
contracting 256 rows per instruction, ~1.5-1.8x bf16 throughput).  The x64
weight scale is undone by the activation `scale` for mm1 and folded into the
routing-weight broadcast for mm2.  The router runs in bf16 (its weights feed
a >0.1 threshold gate, so it gets the accurate path).

All device tensors live transposed ([feature, token]) so the contraction dim
is on SBUF partitions for every matmul.  Host prep is layout/dtype only
(transpose + cast + per-expert slicing).
"""

import os
import numpy as np
import ml_dtypes

B, D, E, U = 4096, 1024, 8, 4096
BT = 512              # token tile (matmul free dim)
NB = B // BT          # 8 token tiles
DC = D // 128         # 8 chunks of the model dim
UC = U // 128         # 32 chunks of the hidden dim
N_CORES = 8
P = 128
WSCALE = 64.0         # host pre-scale on W1/W2 so fp8e4 values are ~N(0,1..2)

_BF16 = ml_dtypes.bfloat16
_F8 = ml_dtypes.float8_e4m3   # TRN fp8e4: max normal +-240, then +-inf

_NC_CACHE = {}
LAST_RESULTS = None


def _build_nc(bench_loop=0):
    import concourse.mybir as mybir
    import concourse.tile as tile
    from concourse import bacc

    f32 = mybir.dt.float32
    bf16 = mybir.dt.bfloat16
    f8 = mybir.dt.float8e4
    AF = mybir.ActivationFunctionType
    ALU = mybir.AluOpType
    DR = mybir.MatmulPerfMode.DoubleRow

    nc = bacc.Bacc("TRN2", target_bir_lowering=False, debug=False,
                   num_devices=N_CORES)

    xt = nc.dram_tensor("xt", [D, B], bf16, kind="ExternalInput").ap()
    xt8 = nc.dram_tensor("xt8", [D, B], f8, kind="ExternalInput").ap()
    w1 = nc.dram_tensor("w1", [D, U], f8, kind="ExternalInput").ap()
    w2 = nc.dram_tensor("w2", [U, D], f8, kind="ExternalInput").ap()
    wr = nc.dram_tensor("wr", [D, E], bf16, kind="ExternalInput").ap()
    br = nc.dram_tensor("br", [E, 1], f32, kind="ExternalInput").ap()
    b1 = nc.dram_tensor("b1", [P, UC], f32, kind="ExternalInput").ap()
    b2 = nc.dram_tensor("b2", [P, DC], f32, kind="ExternalInput").ap()
    selb = nc.dram_tensor("selb", [E, P], f32, kind="ExternalInput").ap()
    ones8 = nc.dram_tensor("ones8", [E, 1], f32, kind="ExternalInput").ap()
    ones18 = nc.dram_tensor("ones18", [1, E], f32, kind="ExternalInput").ap()
    o = nc.dram_tensor("o", [D, B], f32, kind="ExternalOutput").ap()

    with tile.TileContext(nc) as tc:
        with (
            tc.tile_pool(name="wp", bufs=1) as wp,
            tc.tile_pool(name="xbp", bufs=17) as xbp,
            tc.tile_pool(name="x8p", bufs=3) as x8p,
            tc.tile_pool(name="hbp", bufs=2) as hbp,
            tc.tile_pool(name="r8p", bufs=4) as r8p,
            tc.tile_pool(name="r1p", bufs=2) as r1p,
            tc.tile_pool(name="scp", bufs=2) as scp,
            tc.tile_pool(name="ctp", bufs=4) as ctp,
            tc.tile_pool(name="ps1p", bufs=2, space="PSUM") as ps1p,
            tc.tile_pool(name="ps2p", bufs=2, space="PSUM") as ps2p,
            tc.tile_pool(name="psrp", bufs=2, space="PSUM") as psrp,
            tc.tile_pool(name="pssp", bufs=1, space="PSUM") as pssp,
        ):
            # fp8 weights as 3D tiles: [p, chunk, free] with row = chunk*128+p,
            # so [:, k:k+2, m] is the DoubleRow [Ki, Ko=2, dim] weight AP
            w1t = wp.tile([P, DC, U], f8)
            w2t = wp.tile([P, UC, D], f8)
            wrt = wp.tile([P, DC * E], bf16)
            b1t = wp.tile([P, UC], f32)
            b2t = wp.tile([P, DC], f32)
            brt = wp.tile([E, 1], f32)
            selbt = wp.tile([E, P], f32)
            o8t = wp.tile([E, 1], f32)
            o18t = wp.tile([1, E], f32)

            def emit_xb(bt):
                b0 = bt * BT
                xbs = []
                for dc in range(DC):
                    xb = xbp.tile([P, BT], bf16, tag="xb")
                    nc.sync.dma_start(
                        out=xb[:], in_=xt[dc * P : (dc + 1) * P, b0 : b0 + BT]
                    )
                    xbs.append(xb)
                x8 = x8p.tile([P, DC, BT], f8, tag="x8")
                for dc in range(DC):
                    nc.sync.dma_start(
                        out=x8[:, dc, :],
                        in_=xt8[dc * P : (dc + 1) * P, b0 : b0 + BT],
                    )
                return xbs, x8

            def emit_router(xbs):
                # weights = softmax(softmax(x@Wr + br)), gate >0.1, row e
                # broadcast to 128 partitions (scaled by 1/WSCALE for mm2)
                lg = psrp.tile([E, BT], f32, tag="rps")
                for dc in range(DC):
                    nc.tensor.matmul(
                        lg[:], wrt[:, dc * E : (dc + 1) * E], xbs[dc][:],
                        start=(dc == 0), stop=(dc == DC - 1),
                    )
                t1 = r8p.tile([E, BT], f32, tag="r8")
                nc.scalar.activation(t1[:], lg[:], AF.Exp, bias=brt[:, 0:1])
                s1 = psrp.tile([1, BT], f32, tag="rps")
                nc.tensor.matmul(s1[:], o8t[:], t1[:], start=True, stop=True)
                r1 = r1p.tile([1, BT], f32, tag="r1")
                nc.vector.reciprocal(r1[:], s1[:])
                rb1 = psrp.tile([E, BT], f32, tag="rps")
                nc.tensor.matmul(rb1[:], o18t[:], r1[:], start=True, stop=True)
                pp = r8p.tile([E, BT], f32, tag="r8")
                nc.vector.tensor_tensor(pp[:], t1[:], rb1[:], ALU.mult)
                t2 = r8p.tile([E, BT], f32, tag="r8")
                nc.scalar.activation(t2[:], pp[:], AF.Exp)
                s2 = psrp.tile([1, BT], f32, tag="rps")
                nc.tensor.matmul(s2[:], o8t[:], t2[:], start=True, stop=True)
                r2 = r1p.tile([1, BT], f32, tag="r1")
                nc.vector.reciprocal(r2[:], s2[:])
                rb2 = psrp.tile([E, BT], f32, tag="rps")
                nc.tensor.matmul(rb2[:], o18t[:], r2[:], start=True, stop=True)
                wg = r8p.tile([E, BT], f32, tag="r8")
                nc.vector.tensor_tensor(wg[:], t2[:], rb2[:], ALU.mult)
                sc = r8p.tile([E, BT], f32, tag="r8")
                nc.vector.scalar_tensor_tensor(
                    sc[:], wg[:], 0.1, wg[:], ALU.is_gt, ALU.mult
                )
                s128ps = pssp.tile([P, BT], f32, tag="pss")
                nc.tensor.matmul(s128ps[:], selbt[:], sc[:], start=True, stop=True)
                s128 = scp.tile([P, BT], f32, tag="s128")
                nc.vector.tensor_copy(s128[:], s128ps[:])
                return s128

            # small/const DMAs first so the bt=0 router isn't queued behind
            # the 8.4MB of weights
            nc.sync.dma_start(out=b1t[:], in_=b1[:])
            nc.sync.dma_start(out=b2t[:], in_=b2[:])
            nc.sync.dma_start(out=brt[:], in_=br[:])
            nc.sync.dma_start(out=selbt[:], in_=selb[:])
            nc.sync.dma_start(out=o8t[:], in_=ones8[:])
            nc.sync.dma_start(out=o18t[:], in_=ones18[:])
            for dc in range(DC):
                nc.sync.dma_start(
                    out=wrt[:, dc * E : (dc + 1) * E],
                    in_=wr[dc * P : (dc + 1) * P, :],
                )
            def emit_main(prologue=None):
                if prologue is None:
                    xbs_cur, x8_cur = emit_xb(0)
                    s128_cur = emit_router(xbs_cur)
                else:
                    xbs_cur, x8_cur, s128_cur = prologue
                for bt in range(NB):
                    b0 = bt * BT
                    x8 = x8_cur
                    s128 = s128_cur

                    # ---- h^T = swish((W1*64)^T x^T / 64 + b1), fp8 DoubleRow
                    h8 = hbp.tile([P, UC, BT], f8, tag="hb")
                    for uc in range(UC):
                        ps1 = ps1p.tile([P, BT], f32, tag="ps1")
                        for dk in range(DC // 2):
                            nc.tensor.matmul(
                                ps1[:],
                                w1t[:, 2 * dk : 2 * dk + 2,
                                    uc * P : (uc + 1) * P],
                                x8[:, 2 * dk : 2 * dk + 2, :],
                                start=(dk == 0), stop=(dk == DC // 2 - 1),
                                perf_mode=DR,
                            )
                        nc.scalar.activation(h8[:, uc, :], ps1[:], AF.Silu,
                                             bias=b1t[:, uc : uc + 1],
                                             scale=1.0 / WSCALE)

                    # prefetch + route the next token tile while this computes
                    if bt + 1 < NB:
                        xbs_cur, x8_cur = emit_xb(bt + 1)
                        s128_cur = emit_router(xbs_cur)

                    # ---- contrib^T = ((W2*64)^T h^T + 64*b2) * (w*mask/64)
                    for dc in range(DC):
                        ps2 = ps2p.tile([P, BT], f32, tag="ps2")
                        for uk in range(UC // 2):
                            nc.tensor.matmul(
                                ps2[:],
                                w2t[:, 2 * uk : 2 * uk + 2,
                                    dc * P : (dc + 1) * P],
                                h8[:, 2 * uk : 2 * uk + 2, :],
                                start=(uk == 0), stop=(uk == UC // 2 - 1),
                                perf_mode=DR,
                            )
                        ct = ctp.tile([P, BT], f32, tag="ct")
                        nc.vector.scalar_tensor_tensor(
                            ct[:], ps2[:], b2t[:, dc : dc + 1], s128[:],
                            ALU.add, ALU.mult,
                        )
                        nc.sync.dma_start(
                            out=o[dc * P : (dc + 1) * P, b0 : b0 + BT], in_=ct[:]
                        )

            if not bench_loop:
                xbs0, x80 = emit_xb(0)
                s1280 = emit_router(xbs0)

            # W1 in uc-consumption order (g-major) so mm1 can start early;
            # W2 after (first needed ~50us in)
            for g in range(4):
                for dc in range(DC):
                    nc.sync.dma_start(
                        out=w1t[:, dc, g * 1024 : (g + 1) * 1024],
                        in_=w1[dc * P : (dc + 1) * P, g * 1024 : (g + 1) * 1024],
                    )
            for uc in range(UC):
                nc.sync.dma_start(
                    out=w2t[:, uc, :],
                    in_=w2[uc * P : (uc + 1) * P, :],
                )

            if bench_loop:
                with tc.For_i(0, bench_loop, 1):
                    emit_main()
            else:
                emit_main(prologue=(xbs0, x80, s1280))

    nc.compile()
    return nc


def _get_nc():
    if "nc" not in _NC_CACHE:
        _NC_CACHE["nc"] = _build_nc()
    return _NC_CACHE["nc"]


def _f8(a):
    return np.clip(a, -240.0, 240.0).astype(_F8)


def _prep_in_maps(inputs):
    x = np.asarray(inputs["x"], np.float32)
    Wr = np.asarray(inputs["Wr"], np.float32)
    br = np.asarray(inputs["br"], np.float32)
    W1 = np.asarray(inputs["W1"], np.float32)
    b1 = np.asarray(inputs["b1"], np.float32)
    W2 = np.asarray(inputs["W2"], np.float32)
    b2 = np.asarray(inputs["b2"], np.float32)

    xT = np.ascontiguousarray(x.T)            # [D, B] f32
    xt_bf = xT.astype(_BF16)
    xt_f8 = _f8(xT)
    wr_bf = np.ascontiguousarray(Wr).astype(_BF16)
    br_c = np.ascontiguousarray(br.reshape(E, 1))
    ones8_c = np.ones((E, 1), np.float32)
    ones18_c = np.ones((1, E), np.float32)

    in_maps = []
    for c in range(N_CORES):
        sel = np.zeros((E, P), np.float32)
        sel[c, :] = 1.0 / WSCALE
        in_maps.append({
            "xt": xt_bf,
            "xt8": xt_f8,
            "w1": _f8(np.ascontiguousarray(W1[c]) * WSCALE),
            "w2": _f8(np.ascontiguousarray(W2[c]) * WSCALE),
            "wr": wr_bf,
            "br": br_c,
            "b1": np.ascontiguousarray(b1[c].reshape(UC, P).T),
            "b2": np.ascontiguousarray(b2[c].reshape(DC, P).T) * WSCALE,
            "selb": sel,
            "ones8": ones8_c,
            "ones18": ones18_c,
        })
    return in_maps


def kernel(**inputs):
    from concourse.bass_utils import run_bass_kernel_spmd

    global LAST_RESULTS

    in_maps = _prep_in_maps(inputs)
    nc = _get_nc()
    want_trace = bool(int(os.environ.get("KERNEL_TRACE", "0")))
    if not want_trace:
        # the NTFF-trace path needs antenv.axon_hooks, which this container
        # lacks; make sure a stray BASS_TRACE env can't route us into it
        os.environ["BASS_NEVER_TRACE"] = "1"
    res = run_bass_kernel_spmd(
        nc, in_maps, core_ids=list(range(N_CORES)), trace=want_trace,
    )
    LAST_RESULTS = res

    # host: 8-way partial-sum reduction + residual + transpose back
    acc = res.results[0]["o"].astype(np.float32, copy=True)
    for c in range(1, N_CORES):
        acc += res.results[c]["o"]
    out = acc.T + np.asarray(inputs["x"], np.float32)
    return np.ascontiguousarray(out)
